# revision 1
# baseline (speedup 1.0000x reference)
"""Single-head causal attention on 8 Trainium2 NeuronCores (Bass/Tile).

Reference: q = x@wq, k = x@wk, v = x@wv  (x: [32, 768, 256], w*: [256, 64])
           out = softmax(causal(q k^T / 8)) @ v        -> [32, 768, 64]

Sharding: data-parallel over batch, 4 samples per core, no collectives.

Design (all matmul operands bf16, fp32 PSUM accumulation; rel err ~4e-3
against the fp32 reference, well inside the 2e-2 gate):
  - x is pre-transposed and pre-cast to bf16 on the host: xt[p, k, t] with
    C=256 split as two partition chunks of 128. Halves input DMA bytes and
    makes every matmul full-rate at any width (bf16 = 1 col/cycle; fp32r
    drops to 1/4 rate below 256 columns, which crippled the old PV stage).
    bf16 stationaries also enable Fast Weight Load (2 elems/cycle).
  - qT/kT [64, 768] = w^T @ xT with the weights stationary (2 LDW each via
    chunk-outer ordering); both drain PSUM->SBUF through one shared 2-bank
    PSUM region (q first, k reuses it after the q copy).
  - v natural [768, 64] computed directly with xT blocks stationary
    (12 small matmuls) -- no PE transposes. Two ones-columns appended so
    the PV matmul also emits softmax row sums.
  - scoresT rows (keys-block r on partitions) land in a 4-bank PSUM region
    (two 1024-aligned halves, rows alternate); ScalarE exp drains each row
    into e_sb [128, 6, 768] bf16 (one act per row, fused cast + 1/8 scale).
  - causal: only lower-triangular row segments are computed; the diagonal
    128x128 blocks sit at e_sb[:, r, 0:128], so one broadcast multiply with
    a 0/1 triangular mask handles all six of them.
  - PV: e-block stationary [128,128] (FWL), v moving [128, 66]; fp32 PSUM
    accumulation per 128-query block; reciprocal of the row-sum column and
    one multiply produce the bf16 output tile.
  - Two-deep software pipeline: slot b runs proj/v of sample b, the score
    rows + exp of sample b-1 spread across the whole slot (so ScalarE never
    blocks the PE), and the PV of sample b-2 interleaved between rows.
  - Output is written as bf16 and cast to fp32 on the host.
  - For timing runs the hw loop wraps 8 unrolled bodies per iteration:
    the pipeline drain/refill at a loop-boundary costs ~14 us, and the
    Tile cross-body buffer rotation lets unrolled bodies overlap.

PSUM budget (8 banks): qk shared region 2 + scores halves 4 + v 1 + pv 1.

Infrastructure notes: this walrus build accepts at most ONE sync-wait per
instruction, so a post-pass hoists extra waits onto same-engine NoOps.
ScalarE runs only Exp (table reloads cost ~2.7 us).
"""
import numpy as np

import bass_rust
import concourse.bass as bass
import concourse.mybir as mybir
import concourse.tile as tile
from concourse.bass_utils import run_bass_kernel_spmd

F32 = mybir.dt.float32
BF16 = mybir.dt.bfloat16

N_CORES = 8
B, T, C, H = 32, 768, 256, 64
BPC = B // N_CORES  # samples per core
NJ = T // 128  # 128-wide key/query blocks per sample
SCALE = 1.0 / np.sqrt(H)


# --- workaround: this walrus build rejects instructions carrying more than
# one sync-wait command. Tile emits multi-waits freely (joins, final drain).
# Legalize post-hoc: hoist all but the last wait of each instruction onto
# same-engine NoOps inserted just before it (per-engine program order makes
# this semantically identical).
def _legalize_waits(nc):
    n_fix = 0
    for f in nc.m.functions:
        for bb in f.blocks:
            out = []
            for ins in bb.instructions:
                si = ins.sync_info
                if si is not None and si.on_wait and len(si.on_wait) > 1:
                    waits = list(si.on_wait)
                    for w in waits[:-1]:
                        nop = mybir.InstNoOp(
                            name=f"waitfix-{n_fix}", engine=ins.engine
                        )
                        nop.sync_info = bass_rust.SyncInfo(
                            on_wait=[w], on_update=[]
                        )
                        out.append(nop)
                        n_fix += 1
                    si.on_wait = [waits[-1]]
                out.append(ins)
            bb.instructions[:] = out
    return n_fix


def _bank_spans(width):
    """Split [0, width) into matmul spans that stay inside 512-col PSUM
    banks: (0,512),(512,256) etc."""
    out = []
    o = 0
    while o < width:
        w = min(512 - (o % 512), width - o)
        out.append((o, w))
        o += w
    return out


class _Emitter:
    """Holds tiles + emits the per-sample stage pieces in pipeline order."""

    def __init__(self, nc, consts, xt_d, out_d, pools):
        self.nc = nc
        self.consts = consts
        self.xt_d = xt_d
        self.out_d = out_d
        (self.x_pool, self.qk_pool, self.v_pool, self.e_pool, self.o_pool,
         self.r_pool, self.qk_psp, self.s_psp, self.v_psp, self.pv_psp) = pools

    def new_iter(self):
        nc = self.nc
        self.xt = self.x_pool.tile([128, BPC, 2, T], BF16, tag="xt", name="xt")
        nc.sync.dma_start(self.xt[:], self.xt_d[:].rearrange("b p k t -> p b k t"))
        self.o_all = self.o_pool.tile([128, BPC, NJ, H], BF16, tag="oall", name="oall")
        # per-sample state handed between pipeline stages
        self.q_sb = [None] * BPC
        self.k_sb = [None] * BPC
        self.v_sb = [None] * BPC
        self.e_sb = [None] * BPC
        self.s_ps = [None] * BPC
        self.pv_ps = [None] * BPC

    def flush_iter(self):
        self.nc.sync.dma_start(
            self.out_d[:].rearrange("b (a p) h -> p b a h", p=128), self.o_all[:])

    # ---- front pieces (sample b) -------------------------------------
    def emit_q(self, b):
        nc, (wq_sb, wk_sb, wv_sb, mask_sb, ones_sb) = self.nc, self.consts
        ps = self.qk_psp.tile([64, T], F32, tag="qkps", name="qkps")
        for c in range(2):
            for i0, w in _bank_spans(T):
                nc.tensor.matmul(
                    ps[:, i0 : i0 + w], wq_sb[:, c, :],
                    self.xt[:, b, c, i0 : i0 + w],
                    start=(c == 0), stop=(c == 1))
        self.q_sb[b] = self.qk_pool.tile([64, T], BF16, tag="qsb", name="qsb")
        nc.vector.tensor_copy(self.q_sb[b][:], ps[:])

    def emit_k(self, b):
        nc, (wq_sb, wk_sb, wv_sb, mask_sb, ones_sb) = self.nc, self.consts
        ps = self.qk_psp.tile([64, T], F32, tag="qkps", name="qkps")
        for c in range(2):
            for i0, w in _bank_spans(T):
                nc.tensor.matmul(
                    ps[:, i0 : i0 + w], wk_sb[:, c, :],
                    self.xt[:, b, c, i0 : i0 + w],
                    start=(c == 0), stop=(c == 1))
        self.k_sb[b] = self.qk_pool.tile([64, T], BF16, tag="ksb", name="ksb")
        nc.vector.tensor_copy(self.k_sb[b][:], ps[:])

    def emit_v(self, b):
        nc, (wq_sb, wk_sb, wv_sb, mask_sb, ones_sb) = self.nc, self.consts
        ps = self.v_psp.tile([128, NJ, H], F32, tag="vps", name="vps")
        for tb in range(NJ):
            for c in range(2):
                nc.tensor.matmul(
                    ps[:, tb, :],
                    self.xt[:, b, c, 128 * tb : 128 * tb + 128],
                    wv_sb[:, c, :],
                    start=(c == 0), stop=(c == 1))
        v_sb = self.v_pool.tile([128, NJ, H + 2], BF16, tag="vsb", name="vsb")
        nc.vector.tensor_copy(v_sb[:, :, 0:H], ps[:])
        nc.vector.tensor_copy(
            v_sb[:, :, H : H + 2],
            ones_sb[:].rearrange("p (a b) -> p a b", b=2))
        self.v_sb[b] = v_sb

    # ---- score rows + exp (sample b) ---------------------------------
    def emit_score_row(self, b, r):
        nc, (wq_sb, wk_sb, wv_sb, mask_sb, ones_sb) = self.nc, self.consts
        if r == 0:
            self.s_ps[b] = self.s_psp.tile([128, 2, 1024], F32, tag="sps", name="sps")
            self.e_sb[b] = self.e_pool.tile([128, NJ, T], BF16, tag="esb", name="esb")
        w_r = T - 128 * r
        sp = self.s_ps[b][:, r % 2]
        kT = self.k_sb[b][:, 128 * r : 128 * r + 128]
        for i0, w in _bank_spans(w_r):
            nc.tensor.matmul(
                sp[:, i0 : i0 + w], kT,
                self.q_sb[b][:, 128 * r + i0 : 128 * r + i0 + w],
                start=True, stop=True)
        nc.scalar.activation(
            self.e_sb[b][:, r, 0:w_r], sp[:, 0:w_r],
            mybir.ActivationFunctionType.Exp, scale=float(SCALE))
        if r == NJ - 1:
            nc.vector.tensor_mul(
                self.e_sb[b][:, :, 0:128],
                self.e_sb[b][:, :, 0:128],
                mask_sb[:].rearrange("p (a f) -> p a f", a=1)
                .broadcast_to([128, NJ, 128]))

    # ---- PV pieces (sample b); part in 0..n_parts-1 -------------------
    PV_PAIRS = [(ic, jc) for ic in range(NJ) for jc in range(ic + 1)]
    PV_SPLITS = [0, 7, 14, 21]

    def emit_pv_part(self, b, part):
        nc = self.nc
        if part == 0:
            self.pv_ps[b] = self.pv_psp.tile([128, NJ, H + 2], F32, tag="pvps", name="pvps")
        for ic, jc in self.PV_PAIRS[self.PV_SPLITS[part] : self.PV_SPLITS[part + 1]]:
            nc.tensor.matmul(
                self.pv_ps[b][:, ic, :],
                self.e_sb[b][:, jc, 128 * (ic - jc) : 128 * (ic - jc) + 128],
                self.v_sb[b][:, jc, :],
                start=(jc == 0), stop=(jc == ic))
        if part == len(self.PV_SPLITS) - 2:
            r_sb = self.r_pool.tile([128, NJ], F32, tag="rsb", name="rsb")
            nc.vector.reciprocal(r_sb[:], self.pv_ps[b][:, :, H])
            nc.vector.tensor_mul(
                self.o_all[:, b],
                self.pv_ps[b][:, :, 0:H],
                r_sb[:].rearrange("p (c a) -> p c a", a=1)
                .broadcast_to([128, NJ, H]))


def build(repeats=1, unroll=False):
    """Build the SPMD Bass program. repeats>1 wraps the whole per-core body
    in a hardware loop (for timing); unroll=True replicates the body in
    Python instead (for TimelineSim, which can't follow register branches)."""
    nc = bass.Bass("TRN2", target_bir_lowering=False, debug=False, num_devices=N_CORES)

    xt_d = nc.dram_tensor("xt", [BPC, 128, 2, T], BF16, kind="ExternalInput")
    wq_d = nc.dram_tensor("wq", [128, 2, H], BF16, kind="ExternalInput")
    wk_d = nc.dram_tensor("wk", [128, 2, H], BF16, kind="ExternalInput")
    wv_d = nc.dram_tensor("wv", [128, 2, H], BF16, kind="ExternalInput")
    out_d = nc.dram_tensor("out", [BPC, T, H], BF16, kind="ExternalOutput")

    import ml_dtypes

    mask01 = np.triu(np.ones((128, 128))).astype(ml_dtypes.bfloat16)
    mask_d = nc.inline_tensor(mask01, name="mask01")
    ones_d = nc.inline_tensor(
        np.ones((128, NJ * 2), dtype=ml_dtypes.bfloat16), name="ones")

    with tile.TileContext(nc) as tc:
        with (
            tc.tile_pool(name="const", bufs=1) as cpool,
            tc.tile_pool(name="x", bufs=2) as x_pool,
            tc.tile_pool(name="qk", bufs=2) as qk_pool,
            tc.tile_pool(name="v", bufs=3) as v_pool,
            tc.tile_pool(name="e", bufs=2) as e_pool,
            tc.tile_pool(name="o", bufs=2) as o_pool,
            tc.tile_pool(name="r", bufs=2) as r_pool,
            tc.tile_pool(name="qkps", bufs=1, space=bass.MemorySpace.PSUM) as qk_psp,
            tc.tile_pool(name="sps", bufs=1, space=bass.MemorySpace.PSUM) as s_psp,
            tc.tile_pool(name="vps", bufs=1, space=bass.MemorySpace.PSUM) as v_psp,
            tc.tile_pool(name="pvps", bufs=1, space=bass.MemorySpace.PSUM) as pv_psp,
        ):
            wq_sb = cpool.tile([128, 2, H], BF16)
            wk_sb = cpool.tile([128, 2, H], BF16)
            wv_sb = cpool.tile([128, 2, H], BF16)
            mask_sb = cpool.tile([128, 128], BF16)
            ones_sb = cpool.tile([128, NJ * 2], BF16)
            nc.gpsimd.dma_start(wq_sb[:], wq_d[:])
            nc.gpsimd.dma_start(wk_sb[:], wk_d[:])
            nc.gpsimd.dma_start(wv_sb[:], wv_d[:])
            nc.gpsimd.dma_start(mask_sb[:], mask_d[:])
            nc.gpsimd.dma_start(ones_sb[:], ones_d[:])
            consts = (wq_sb, wk_sb, wv_sb, mask_sb, ones_sb)

            pools = (x_pool, qk_pool, v_pool, e_pool, o_pool, r_pool,
                     qk_psp, s_psp, v_psp, pv_psp)
            em = _Emitter(nc, consts, xt_d, out_d, pools)

            def body():
                em.new_iter()
                # slot plan: F(b) = q/v/k of sample b; S(b-1) = score rows,
                # P(b-2) = PV parts, interleaved to keep every engine fed.
                for slot in range(BPC + 2):
                    fb = slot if slot < BPC else None
                    sb = slot - 1 if 0 <= slot - 1 < BPC else None
                    pb = slot - 2 if 0 <= slot - 2 < BPC else None
                    if fb is not None:
                        em.emit_q(fb)
                    if sb is not None:
                        em.emit_score_row(sb, 0)
                    if fb is not None:
                        em.emit_v(fb)
                    if sb is not None:
                        em.emit_score_row(sb, 1)
                    if fb is not None:
                        em.emit_k(fb)
                    if sb is not None:
                        em.emit_score_row(sb, 2)
                    if pb is not None:
                        em.emit_pv_part(pb, 0)
                    if sb is not None:
                        em.emit_score_row(sb, 3)
                    if pb is not None:
                        em.emit_pv_part(pb, 1)
                    if sb is not None:
                        em.emit_score_row(sb, 4)
                    if pb is not None:
                        em.emit_pv_part(pb, 2)
                    if sb is not None:
                        em.emit_score_row(sb, 5)
                em.flush_iter()

            if repeats == 1:
                body()
            elif unroll:
                for _ in range(repeats):
                    body()
            else:
                # unroll a few bodies per hw-loop iteration so cross-iteration
                # pipeline bubbles amortize
                nb = next(n for n in (8, 4, 2, 1) if repeats % n == 0)
                with tc.For_i(0, repeats // nb, 1, staggered_reset=True):
                    for _ in range(nb):
                        body()
    import os
    if not os.environ.get("SKIP_LEGALIZE"):
        _legalize_waits(nc)
    return nc


def _prep_inputs(x, wq, wk, wv):
    import ml_dtypes

    x = np.asarray(x, dtype=np.float32)
    # xT per sample with c split into 2 partition chunks:
    # xt[b, p, k, t] = x[b, t, 128k + p]
    xt = np.ascontiguousarray(
        x.reshape(B, T, 2, 128).transpose(0, 3, 2, 1)).astype(ml_dtypes.bfloat16)

    def packw(w):
        w = np.asarray(w, dtype=np.float32)
        return np.ascontiguousarray(
            w.reshape(2, 128, H).transpose(1, 0, 2)).astype(ml_dtypes.bfloat16)

    return xt, packw(wq), packw(wk), packw(wv)


_NC_CACHE = {}


def _get_nc(repeats=1):
    if repeats not in _NC_CACHE:
        _NC_CACHE[repeats] = build(repeats)
    return _NC_CACHE[repeats]


def run(x, wq, wk, wv, repeats=1):
    xt, wqp, wkp, wvp = _prep_inputs(x, wq, wk, wv)
    nc = _get_nc(repeats)
    in_maps = [
        {"xt": xt[c * BPC : (c + 1) * BPC], "wq": wqp, "wk": wkp, "wv": wvp}
        for c in range(N_CORES)
    ]
    res = run_bass_kernel_spmd(nc, in_maps, core_ids=list(range(N_CORES)))
    return np.concatenate(
        [res.results[c]["out"].astype(np.float32) for c in range(N_CORES)], axis=0)


def kernel(x, wq, wk, wv):
    return run(x, wq, wk, wv, repeats=1)



# revision 29
# speedup vs baseline: 3.4723x; 3.4723x over previous
"""Single-head causal attention on 8 Trainium2 NeuronCores (Bass/Tile).

Reference: q = x@wq, k = x@wk, v = x@wv  (x: [32, 768, 256], w*: [256, 64])
           out = softmax(causal(q k^T / 8)) @ v        -> [32, 768, 64]

Sharding: data-parallel over batch, 4 samples per core, no collectives.

Design v2 (all matmul operands bf16, fp32 PSUM accumulation):
  - x pre-transposed/pre-cast on host: xt[p, k, t], C=256 as two 128-partition
    chunks. bf16 halves DMA bytes and keeps every matmul full-rate.
  - q and k projections fused: stationary [wq|wk] [128, 128] -> one pass over
    xt yields qkT [128, 768] (rows 0:63 = qT, 64:127 = kT) in a 2-bank PSUM
    region; ONE DVE drain [128, 768] -> SBUF bf16; a SBUF->SBUF DMA then moves
    the k half down to partitions 0:63 (k_sb) so score matmuls can contract
    q (partitions 0:63) against kT blocks.
  - v natural [768, 64] with xt blocks stationary (12 matmuls); two
    ones-columns appended so PV also emits softmax row sums.
  - scores: rows r paired (0,5),(1,4),(2,3); each pair lands in one 2-bank
    PSUM region [128, 896] (row p at col 0, row 5-p at col 768-128p), so ONE
    ScalarE exp per pair (3 activations/sample instead of 6 -- the ~220-cycle
    per-activation overhead was significant) draining into e2_sb[128, 3, 896].
  - causal: only lower-triangular row segments computed; diagonal 128x128
    blocks (2 per pair) masked by GpSimd (Pool) tensor_mul with a 0/1
    triangular mask -- Pool is otherwise idle, freeing DVE.
  - PV: parts regrouped by key-row pair: part k uses exactly rows of exp pair
    k (7 block-pairs each), so pv part k of sample b only needs pair k masked
    one slot earlier. e-block stationary [128,128], v moving [128, 66].
  - Slot pipeline (steady state): P=pv(b-2) / S=scores+exp+mask(b-1) /
    F=proj(b) interleaved so PE, ScalarE (exp, ~2.8us/sample, the pacing
    engine), DVE (drains+normalize ~2.2us) and Pool all stay fed.
  - PSUM budget (8 banks): qk 2 + score pairs 2x2 + v 1 + pv 1.
  - Output bf16, cast to fp32 on host.

Infrastructure notes: this walrus build accepts at most ONE sync-wait per
instruction, so a post-pass hoists extra waits onto same-engine NoOps.
ScalarE runs only Exp (table reloads cost ~2.7 us).
"""
import numpy as np

import bass_rust
import concourse.bass as bass
import concourse.mybir as mybir
import concourse.tile as tile
from concourse.bass_utils import run_bass_kernel_spmd

F32 = mybir.dt.float32
BF16 = mybir.dt.bfloat16

N_CORES = 8
B, T, C, H = 32, 768, 256, 64
BPC = B // N_CORES  # samples per core
NJ = T // 128  # 128-wide key/query blocks per sample
SCALE = 1.0 / np.sqrt(H)

# score-row pairs: pair p = rows (p, 5-p); row p stored at col 0 (width
# 768-128p), row 5-p at col 768-128p (width 128+128p); total 896 per pair.
PAIR_W = [768 - 128 * p for p in range(3)]


def _row_loc(r):
    """(pair, base_col) of score row r in e2/psum pair storage."""
    p = min(r, 5 - r)
    return p, (0 if r == p else PAIR_W[p])


# --- workaround: this walrus build rejects instructions carrying more than
# one sync-wait command. Tile emits multi-waits freely (joins, final drain).
# Legalize post-hoc: hoist all but the last wait of each instruction onto
# same-engine NoOps inserted just before it (per-engine program order makes
# this semantically identical).
def _legalize_waits(nc):
    n_fix = 0
    for f in nc.m.functions:
        for bb in f.blocks:
            out = []
            for ins in bb.instructions:
                si = ins.sync_info
                if si is not None and si.on_wait and len(si.on_wait) > 1:
                    waits = list(si.on_wait)
                    for w in waits[:-1]:
                        nop = mybir.InstNoOp(
                            name=f"waitfix-{n_fix}", engine=ins.engine
                        )
                        nop.sync_info = bass_rust.SyncInfo(
                            on_wait=[w], on_update=[]
                        )
                        out.append(nop)
                        n_fix += 1
                    si.on_wait = [waits[-1]]
                # the 64B ISA encoding shares one immediate between
                # wait_value and update_value: a sem-ge-imm wait plus a
                # sem-add-imm update on one instruction fails walrus's
                # 'no_semaphore_value_conflict' check. Hoist the wait onto a
                # NoOp in front (same-engine program order keeps semantics).
                if (
                    si is not None
                    and si.on_wait
                    and any(u.update_mode == "sem-add-imm" for u in si.on_update)
                    and any(w.wait_mode == "sem-ge-imm" for w in si.on_wait)
                ):
                    nop = mybir.InstNoOp(
                        name=f"waitfix-{n_fix}", engine=ins.engine
                    )
                    nop.sync_info = bass_rust.SyncInfo(
                        on_wait=list(si.on_wait), on_update=[]
                    )
                    out.append(nop)
                    n_fix += 1
                    si.on_wait = []
                out.append(ins)
            bb.instructions[:] = out
    return n_fix


def _bank_spans(width, offset=0):
    """Split [offset, offset+width) into spans that stay inside 512-col PSUM
    banks."""
    out = []
    o = offset
    end = offset + width
    while o < end:
        w = min(512 - (o % 512), end - o)
        out.append((o, w))
        o += w
    return out


class _Emitter:
    """Holds tiles + emits the per-sample stage pieces in pipeline order."""

    # PV block-pairs in query-block-contiguous order: each psum block ic's
    # accumulation group (over jc) must be contiguous among matmuls targeting
    # the pv bank -- a start=True from another block clears the whole bank's
    # has_written bits and corrupts open groups. All e-pairs of sample b are
    # exp'd+masked one slot before PV(b) runs, so ordering is otherwise free.
    PV_PAIRS = [(ic, jc) for ic in range(NJ) for jc in range(ic + 1)]
    PV_SPLITS = [0, 7, 14, 21]

    def __init__(self, nc, consts, xt_d, out_d, pools):
        self.nc = nc
        self.consts = consts
        self.xt_d = xt_d
        self.out_d = out_d
        (self.x_pool, self.qk_pool, self.k_pool, self.v_pool, self.e_pool,
         self.o_pool, self.r_pool, self.qk_psp, self.s_psp, self.v_psp,
         self.pv_psp) = pools

    def new_iter(self, first=False):
        nc = self.nc
        # xt prefetch runs one full body ahead: the tile DMA'd at the head of
        # body n is consumed by body n+1, so the ~4.4us HBM load never gates
        # a body start. xt loads + out flushes ride the Activation HWDGE
        # queue (triggers placed where the ACT sequencer never blocks);
        # k-moves get the SP HWDGE queue to themselves so they always land
        # within their issue slot.
        if first:
            self.xt = self.x_pool.tile([128, BPC, 2, T], BF16, tag="xt", name="xt")
            nc.sync.dma_start(self.xt[:], self.xt_d[:].rearrange("b p k t -> p b k t"))
        else:
            self.xt = self.xt_next
        self.xt_next = self.x_pool.tile([128, BPC, 2, T], BF16, tag="xt", name="xt")
        # o(n-1) is completed by the P stages running during body n (lag-4);
        # o(n-2) is flushed at body n's head, when its norms are long done.
        # o_pp's flush trigger is emitted in slot 1 (emit_flush_pp): at the
        # body head its wait (last norm of body n-2, DVE phase-lagged into
        # this body) would block the ACT sequencer's exp stream.
        self.o_pp = getattr(self, "o_prev", None)
        self.o_prev = getattr(self, "o_cur", None)
        self.o_cur = self.o_pool.tile([128, BPC, NJ, H], BF16, tag="oall", name="oall")
        # per-sample state handed between pipeline stages; entries for
        # samples of the previous body stay live until overwritten (the
        # cross-body software pipeline reads them up to 4 slots later)
        if not hasattr(self, "qk_sb"):
            self.qk_sb = [None] * BPC
            self.k_sb = [None] * BPC
            self.v_sb = [None] * BPC
            self.e_sb = [None] * BPC
            self.pv_ps = [None] * BPC

    def emit_xt_prefetch(self):
        # emitted between km(1) and km(2) on the SP queue: early enough that
        # the ~4.4us transfer completes before the next body needs it, late
        # enough that km(0)/km(1) aren't stuck behind it in the queue FIFO
        self.nc.sync.dma_start(
            self.xt_next[:], self.xt_d[:].rearrange("b p k t -> p b k t"))

    def emit_flush_pp(self):
        if self.o_pp is not None:
            self._flush(self.o_pp)
            self.o_pp = None

    def _flush(self, o_tile):
        self.nc.sync.dma_start(
            self.out_d[:].rearrange("b (a p) h -> p b a h", p=128), o_tile[:])

    # ---- front piece (sample b): fused q+k projection + v ------------
    def emit_qk(self, b):
        nc, (wqk_sb, wv_sb, mask_sb, ones_sb) = self.nc, self.consts
        ps = self.qk_psp.tile([128, 1024], F32, tag="qkps", name="qkps")
        for c in range(2):
            for i0, w in _bank_spans(T):
                nc.tensor.matmul(
                    ps[:, i0 : i0 + w], wqk_sb[:, c, :],
                    self.xt[:, b, c, i0 : i0 + w],
                    start=(c == 0), stop=(c == 1))
        self.qk_sb[b] = self.qk_pool.tile([128, T], BF16, tag="qksb", name="qksb")
        nc.vector.tensor_copy(self.qk_sb[b][:], ps[:, 0:T])

    def emit_kmove(self, b):
        # move kT (partitions 64:128) down to partitions 0:63 so score
        # matmuls can use it as stationary against q on partitions 0:63.
        # On the otherwise-empty SP HWDGE queue; S(b) runs 2 slots after the
        # qk drain, so the move has a full slot of latency slack.
        self.k_sb[b] = self.k_pool.tile([64, T], BF16, tag="ksb", name="ksb")
        self.nc.sync.dma_start(self.k_sb[b][:], self.qk_sb[b][64:128, :])

    def emit_v(self, b):
        nc, (wqk_sb, wv_sb, mask_sb, ones_sb) = self.nc, self.consts
        ps = self.v_psp.tile([128, NJ, H], F32, tag="vps", name="vps")
        for tb in range(NJ):
            for c in range(2):
                nc.tensor.matmul(
                    ps[:, tb, :],
                    self.xt[:, b, c, 128 * tb : 128 * tb + 128],
                    wv_sb[:, c, :],
                    start=(c == 0), stop=(c == 1))
        v_sb = self.v_pool.tile([128, NJ, H + 2], BF16, tag="vsb", name="vsb")
        nc.vector.tensor_copy(v_sb[:, :, 0:H], ps[:])
        nc.vector.tensor_copy(
            v_sb[:, :, H : H + 2],
            ones_sb[:].rearrange("p (a b) -> p a b", b=2))
        self.v_sb[b] = v_sb

    # ---- score pair p (sample b): matmuls + one exp + Pool masks ------
    def emit_score_pair(self, b, p):
        nc, (wqk_sb, wv_sb, mask_sb, ones_sb) = self.nc, self.consts
        if p == 0:
            self.e_sb[b] = self.e_pool.tile([128, 3, 896], BF16, tag="esb", name="esb")
        sp = self.s_psp.tile([128, 1024], F32, tag="sps", name="sps")
        qk = self.qk_sb[b]
        for r in (p, 5 - p):
            _, base = _row_loc(r)
            w_r = T - 128 * r
            kT = self.k_sb[b][:, 128 * r : 128 * r + 128]
            for i0, w in _bank_spans(w_r, base):
                qa = 128 * r + (i0 - base)
                nc.tensor.matmul(
                    sp[:, i0 : i0 + w], kT, qk[0:64, qa : qa + w],
                    start=True, stop=True)
        nc.scalar.activation(
            self.e_sb[b][:, p, 0:896], sp[:, 0:896],
            mybir.ActivationFunctionType.Exp, scale=float(SCALE))
        # mask the two diagonal 128x128 blocks of this pair on Pool
        for r in (p, 5 - p):
            _, base = _row_loc(r)
            seg = self.e_sb[b][:, p, base : base + 128]
            nc.gpsimd.tensor_mul(seg, seg, mask_sb[:])
            if r == 5 - p:
                break  # p == 5-p impossible here (NJ=6), but keep safe

    def _e_block(self, b, ic, jc):
        p, base = _row_loc(jc)
        off = base + 128 * (ic - jc)
        return self.e_sb[b][:, p, off : off + 128]

    # ---- PV part k (sample b): block-pairs using key-row pair k -------
    def emit_pv_part(self, b, part):
        nc = self.nc
        if part == 0:
            self.pv_ps[b] = self.pv_psp.tile([128, NJ, H + 2], F32, tag="pvps", name="pvps")
        for ic, jc in self.PV_PAIRS[self.PV_SPLITS[part] : self.PV_SPLITS[part + 1]]:
            nc.tensor.matmul(
                self.pv_ps[b][:, ic, :],
                self._e_block(b, ic, jc),
                self.v_sb[b][:, jc, :],
                start=(jc == 0), stop=(jc == ic))

    def emit_norm(self, b, o_dst):
        nc = self.nc
        r_sb = self.r_pool.tile([128, NJ], F32, tag="rsb", name="rsb")
        nc.vector.reciprocal(r_sb[:], self.pv_ps[b][:, :, H])
        nc.vector.tensor_mul(
            o_dst[:, b],
            self.pv_ps[b][:, :, 0:H],
            r_sb[:].rearrange("p (c a) -> p c a", a=1)
            .broadcast_to([128, NJ, H]))


def build(repeats=1, unroll=False):
    """Build the SPMD Bass program. repeats>1 wraps the whole per-core body
    in a hardware loop (for timing); unroll=True replicates the body in
    Python instead (for TimelineSim, which can't follow register branches)."""
    nc = bass.Bass("TRN2", target_bir_lowering=False, debug=False, num_devices=N_CORES)

    xt_d = nc.dram_tensor("xt", [BPC, 128, 2, T], BF16, kind="ExternalInput")
    wqk_d = nc.dram_tensor("wqk", [128, 2, 128], BF16, kind="ExternalInput")
    wv_d = nc.dram_tensor("wv", [128, 2, H], BF16, kind="ExternalInput")
    out_d = nc.dram_tensor("out", [BPC, T, H], BF16, kind="ExternalOutput")

    import ml_dtypes

    mask01 = np.triu(np.ones((128, 128))).astype(ml_dtypes.bfloat16)
    mask_d = nc.inline_tensor(mask01, name="mask01")
    ones_d = nc.inline_tensor(
        np.ones((128, NJ * 2), dtype=ml_dtypes.bfloat16), name="ones")

    with tile.TileContext(nc) as tc:
        with (
            tc.tile_pool(name="const", bufs=1) as cpool,
            tc.tile_pool(name="x", bufs=2) as x_pool,
            tc.tile_pool(name="qk", bufs=8) as qk_pool,
            tc.tile_pool(name="k", bufs=8) as k_pool,
            tc.tile_pool(name="v", bufs=8) as v_pool,
            tc.tile_pool(name="e", bufs=8) as e_pool,
            tc.tile_pool(name="o", bufs=4) as o_pool,
            tc.tile_pool(name="r", bufs=2) as r_pool,
            tc.tile_pool(name="qkps", bufs=1, space=bass.MemorySpace.PSUM) as qk_psp,
            tc.tile_pool(name="sps", bufs=2, space=bass.MemorySpace.PSUM) as s_psp,
            tc.tile_pool(name="vps", bufs=1, space=bass.MemorySpace.PSUM) as v_psp,
            tc.tile_pool(name="pvps", bufs=1, space=bass.MemorySpace.PSUM) as pv_psp,
        ):
            wqk_sb = cpool.tile([128, 2, 128], BF16)
            wv_sb = cpool.tile([128, 2, H], BF16)
            mask_sb = cpool.tile([128, 128], BF16)
            ones_sb = cpool.tile([128, NJ * 2], BF16)
            nc.gpsimd.dma_start(wqk_sb[:], wqk_d[:])
            nc.gpsimd.dma_start(wv_sb[:], wv_d[:])
            nc.gpsimd.dma_start(mask_sb[:], mask_d[:])
            nc.gpsimd.dma_start(ones_sb[:], ones_d[:])
            consts = (wqk_sb, wv_sb, mask_sb, ones_sb)

            pools = (x_pool, qk_pool, k_pool, v_pool, e_pool, o_pool, r_pool,
                     qk_psp, s_psp, v_psp, pv_psp)
            em = _Emitter(nc, consts, xt_d, out_d, pools)

            def body(first=False):
                # Cross-body software pipeline, lag 2 per stage: slot s runs
                # F(s) = proj of sample s, S(s-2) = score pairs + exp + mask,
                # P(s-4) = PV + norm. Negative stage indices reach into the
                # PREVIOUS body's samples, so the pipeline never drains at a
                # body boundary, and every cross-stage handoff (qk drain ->
                # k-move -> score matmul; exp -> mask -> PV) has a full slot
                # of latency slack. PE order inside a slot paces the three
                # score-pair matmul groups so ScalarE's exp stream (the
                # bottleneck, ~2.8us/slot) never starves.
                em.new_iter(first=first)
                for slot in range(BPC):
                    fb = slot
                    sb = slot - 2 if (slot >= 2 or not first) else None
                    pb = slot - 4 if not first else None
                    if sb is not None:
                        em.emit_score_pair(sb % BPC, 0)
                        em.emit_score_pair(sb % BPC, 1)
                    em.emit_qk(fb)
                    if sb is not None:
                        em.emit_score_pair(sb % BPC, 2)
                    em.emit_v(fb)
                    if pb is not None:
                        em.emit_pv_part(pb % BPC, 0)
                        em.emit_pv_part(pb % BPC, 1)
                        em.emit_pv_part(pb % BPC, 2)
                        em.emit_norm(pb % BPC, em.o_prev)
                    em.emit_kmove(fb)
                    if slot == 1:
                        em.emit_xt_prefetch()
                    if slot == 3:
                        em.emit_flush_pp()

            def epilogue():
                em.emit_flush_pp()
                for sb in (BPC - 2, BPC - 1):
                    for p in range(3):
                        em.emit_score_pair(sb, p)
                if em.o_prev is not None:
                    em._flush(em.o_prev)
                for pb in range(BPC):
                    for part in range(3):
                        em.emit_pv_part(pb, part)
                    em.emit_norm(pb, em.o_cur)
                em._flush(em.o_cur)

            if repeats == 1:
                body(first=True)
                epilogue()
            elif unroll:
                body(first=True)
                for _ in range(repeats - 1):
                    body()
                epilogue()
            else:
                # steady-state bodies inside a hw loop; warmup body, loop
                # remainder bodies, and the pipeline drain emitted in Python
                nb = 8
                n_loop = (repeats - 1) // nb
                n_trail = (repeats - 1) % nb
                body(first=True)
                if n_loop > 0:
                    with tc.For_i(0, n_loop, 1, staggered_reset=True):
                        for _ in range(nb):
                            body()
                for _ in range(n_trail):
                    body()
                epilogue()
    import os
    if not os.environ.get("SKIP_LEGALIZE"):
        _legalize_waits(nc)
    return nc


def _prep_inputs(x, wq, wk, wv):
    import ml_dtypes

    x = np.asarray(x, dtype=np.float32)
    # xT per sample with c split into 2 partition chunks:
    # xt[b, p, k, t] = x[b, t, 128k + p]
    xt = np.ascontiguousarray(
        x.reshape(B, T, 2, 128).transpose(0, 3, 2, 1)).astype(ml_dtypes.bfloat16)

    wq = np.asarray(wq, dtype=np.float32).reshape(2, 128, H)
    wk = np.asarray(wk, dtype=np.float32).reshape(2, 128, H)
    wqk = np.concatenate([wq, wk], axis=2)  # [2, 128, 128]
    wqk = np.ascontiguousarray(wqk.transpose(1, 0, 2)).astype(ml_dtypes.bfloat16)

    wv = np.asarray(wv, dtype=np.float32)
    wvp = np.ascontiguousarray(
        wv.reshape(2, 128, H).transpose(1, 0, 2)).astype(ml_dtypes.bfloat16)
    return xt, wqk, wvp


_NC_CACHE = {}


def _get_nc(repeats=1):
    if repeats not in _NC_CACHE:
        _NC_CACHE[repeats] = build(repeats)
    return _NC_CACHE[repeats]


def run(x, wq, wk, wv, repeats=1):
    xt, wqkp, wvp = _prep_inputs(x, wq, wk, wv)
    nc = _get_nc(repeats)
    in_maps = [
        {"xt": xt[c * BPC : (c + 1) * BPC], "wqk": wqkp, "wv": wvp}
        for c in range(N_CORES)
    ]
    res = run_bass_kernel_spmd(nc, in_maps, core_ids=list(range(N_CORES)))
    return np.concatenate(
        [res.results[c]["out"].astype(np.float32) for c in range(N_CORES)], axis=0)


def kernel(x, wq, wk, wv):
    return run(x, wq, wk, wv, repeats=1)
